# revision 11
# baseline (speedup 1.0000x reference)
"""Trainium2 Bass kernel for nn_AtomicConvScore (MoE-routing style).

Strategy (routed / expert-grouped, data-parallel over atoms):
  * Concatenate frag1/frag2/complex atoms into one list with a per-atom
    sign (+1 complex, -1 frags); the answer is
        out[b] = sum_n sign[n] * MLP_{z[n]}(x[n])   (+ bout correction)
  * Sort atoms by type on the host, pad each type group to a multiple of
    8*128 and give each core 1/8 of every type group -> every core runs
    the IDENTICAL instruction schedule (SPMD) on different data.
  * On device, activations stay feature-major ([feature, atom] layout) so
    no transposes are ever needed:
        h^T = relu(W^T_chunk @ x^T_chunk + b)
  * The per-batch masked sum is a tiny matmul against a host-built
    signed-onehot matrix S[n, b] = sign[n] * (batch[n] == b), accumulated
    in a single PSUM tile across the whole kernel.
  * Host sums the 8 per-core partials and adds the (input-dependent but
    tiny) bout correction term.
"""

import os
import sys

sys.path.insert(0, "/opt/trn_rl_repo")

import numpy as np
import ml_dtypes

import concourse.bass as bass
import concourse.tile as tile
from concourse import bacc, mybir
from concourse.bass_utils import run_bass_kernel_spmd

# Problem constants (hardcoded per the self-contained-kernel contract).
B = 16
F = 256
H1, H2, H3 = 256, 256, 128
T = 5
NCORES = 8
PTILE = 128  # atoms per tile (partition dim)
SUPER = 4    # tiles per supertile -> moving dim N = 512

# Matmul precision mode: "bf16" (1 cyc/row), "f32r" (1 cyc/row @ N>=256),
# "f32" (4 cyc/row, exact).
MM_MODE = os.environ.get("KMODE", "f32r")

TRACE = False          # test.py sets this for profiling runs
LAST_RESULTS = None    # test.py reads exec_time_ns from here

_F32 = mybir.dt.float32


def _dtypes():
    # (device storage dtype for x/W/h, host numpy dtype)
    if MM_MODE == "bf16":
        return mybir.dt.bfloat16, ml_dtypes.bfloat16
    if MM_MODE == "f32r":
        # float32r: fp32 bits in memory, PE rounds to ~tf32 and runs at
        # full rate for moving dim >= 256. All matmul operands must be
        # declared float32r end-to-end (BIR verifier requirement).
        return mybir.dt.float32r, np.float32
    assert MM_MODE == "f32"
    return mybir.dt.float32, np.float32


def _build(k_t, n_core):
    """Build the (SPMD-uniform) Bass program for one core."""
    st_dt, _np_dt = _dtypes()
    # The per-atom head (A-matmul) has moving dim 1, which is illegal for
    # float32r (dst free size must be even) — run it in plain fp32 (bf16
    # mode keeps bf16). Exact head is also good for accuracy.
    head_dt = mybir.dt.bfloat16 if MM_MODE == "bf16" else _F32
    ntt = n_core // PTILE
    relu = mybir.ActivationFunctionType.Relu

    def mm(ap):
        return ap

    nc = bacc.Bacc()
    xT_d = nc.dram_tensor("xT", [F, n_core], st_dt, kind="ExternalInput")
    S_d = nc.dram_tensor("S", [PTILE, ntt * B], _F32, kind="ExternalInput")
    W0_d = nc.dram_tensor("W0", [T, F, H1], st_dt, kind="ExternalInput")
    W1_d = nc.dram_tensor("W1", [T, H1, H2], st_dt, kind="ExternalInput")
    W2_d = nc.dram_tensor("W2", [T, H2, H3], st_dt, kind="ExternalInput")
    Wout_d = nc.dram_tensor("Wout", [T, H3, 1], head_dt, kind="ExternalInput")
    bpack_d = nc.dram_tensor("bpack", [PTILE, T * 5], _F32, kind="ExternalInput")
    out_d = nc.dram_tensor("res", [1, B], _F32, kind="ExternalOutput")

    active = [t for t in range(T) if k_t[t] > 0]
    # supertile schedule: (type, ntiles) with ntiles in 1..SUPER
    sched = []
    for t in active:
        left = k_t[t]
        while left > 0:
            nt = min(SUPER, left)
            sched.append((t, nt))
            left -= nt
    total_b = sum(nt for _, nt in sched)

    with tile.TileContext(nc) as tc:
        with (
            tc.tile_pool(name="wconst", bufs=1) as wpool,
            tc.tile_pool(name="sconst", bufs=1) as spool,
            tc.tile_pool(name="x", bufs=3) as xpool,
            tc.tile_pool(name="h", bufs=2) as hpool,
            tc.tile_pool(name="o", bufs=2) as opool,
            tc.tile_pool(name="r", bufs=1) as rpool,
            tc.tile_pool(name="pl1", bufs=3, space="PSUM") as pl1,
            tc.tile_pool(name="pl2", bufs=2, space="PSUM") as pl2,
            tc.tile_pool(name="pl3", bufs=1, space="PSUM") as pl3,
            tc.tile_pool(name="po", bufs=1, space="PSUM") as popool,
            tc.tile_pool(name="pres", bufs=1, space="PSUM") as prespool,
        ):
            # ---- constants ----
            w0c, w1c, w2c, woc = {}, {}, {}, {}
            for t in active:
                for k in range(2):
                    for m in range(2):
                        w = wpool.tile([128, 128], st_dt, tag=f"w0_{t}_{k}_{m}")
                        nc.sync.dma_start(
                            w[:], W0_d[t, 128 * k:128 * (k + 1), 128 * m:128 * (m + 1)]
                        )
                        w0c[t, k, m] = w
                        w = wpool.tile([128, 128], st_dt, tag=f"w1_{t}_{k}_{m}")
                        nc.sync.dma_start(
                            w[:], W1_d[t, 128 * k:128 * (k + 1), 128 * m:128 * (m + 1)]
                        )
                        w1c[t, k, m] = w
                for k in range(2):
                    w = wpool.tile([128, 128], st_dt, tag=f"w2_{t}_{k}")
                    nc.sync.dma_start(w[:], W2_d[t, 128 * k:128 * (k + 1), 0:128])
                    w2c[t, k] = w
                w = wpool.tile([128, 1], head_dt, tag=f"wo_{t}")
                nc.sync.dma_start(w[:], Wout_d[t, :, 0:1])
                woc[t] = w
            S_sb = spool.tile([PTILE, ntt * B], _F32, tag="S")
            nc.sync.dma_start(S_sb[:], S_d[:])
            b_sb = spool.tile([PTILE, T * 5], _F32, tag="b")
            nc.sync.dma_start(b_sb[:], bpack_d[:])

            pres = prespool.tile([1, B], _F32, tag="pres")

            # ---- main loop ----
            col = 0   # atom column offset
            jg = 0    # global tile index
            bi = 0    # B-matmul counter
            for t, nt in sched:
                N = PTILE * nt
                x0 = xpool.tile([128, N], st_dt, tag="x0")
                nc.sync.dma_start(x0[:], xT_d[0:128, col:col + N])
                x1 = xpool.tile([128, N], st_dt, tag="x1")
                nc.sync.dma_start(x1[:], xT_d[128:256, col:col + N])

                def layer(xa, xb, wdict, bias_col, tag):
                    outs = []
                    for m in range(2):
                        p = (pl1 if tag == "h1" else pl2).tile(
                            [128, N], _F32, tag="p" + tag
                        )
                        nc.tensor.matmul(p[:], mm(wdict[t, 0, m][:]), mm(xa[:]),
                                         start=True, stop=False)
                        nc.tensor.matmul(p[:], mm(wdict[t, 1, m][:]), mm(xb[:]),
                                         start=False, stop=True)
                        h = hpool.tile([128, N], st_dt, tag=f"{tag}_{m}")
                        bias = b_sb[:, bias_col + m: bias_col + m + 1]
                        if m == 0:
                            nc.scalar.activation(h[:], p[:], relu, bias=bias)
                        else:
                            nc.vector.tensor_scalar(
                                h[:], p[:], bias, 0.0,
                                mybir.AluOpType.add, mybir.AluOpType.max,
                            )
                        outs.append(h)
                    return outs

                h1 = layer(x0, x1, w0c, t * 5 + 0, "h1")
                h2 = layer(h1[0], h1[1], w1c, t * 5 + 2, "h2")

                p3 = pl3.tile([128, N], _F32, tag="pl3")
                nc.tensor.matmul(p3[:], mm(w2c[t, 0][:]), mm(h2[0][:]),
                                 start=True, stop=False)
                nc.tensor.matmul(p3[:], mm(w2c[t, 1][:]), mm(h2[1][:]),
                                 start=False, stop=True)
                h3 = hpool.tile([128, N], head_dt, tag="h3")
                nc.scalar.activation(h3[:], p3[:], relu,
                                     bias=b_sb[:, t * 5 + 4: t * 5 + 5])

                po = popool.tile([128, nt], _F32, tag="po")
                for j in range(nt):
                    nc.tensor.matmul(po[:, j:j + 1],
                                     mm(h3[:, 128 * j:128 * (j + 1)]),
                                     mm(woc[t][:]), start=True, stop=True)
                o_sb = opool.tile([128, nt], _F32, tag="o")
                nc.vector.tensor_copy(o_sb[:], po[:])
                for j in range(nt):
                    scol = (jg + j) * B
                    nc.tensor.matmul(pres[:], o_sb[:, j:j + 1],
                                     S_sb[:, scol:scol + B],
                                     start=(bi == 0), stop=(bi == total_b - 1),
                                     skip_group_check=True)
                    bi += 1
                jg += nt
                col += N

            res_sb = rpool.tile([1, B], _F32, tag="res")
            nc.scalar.copy(res_sb[:], pres[:])
            nc.sync.dma_start(out_d[:], res_sb[:])
    nc.finalize()
    return nc


def kernel(**inputs):
    global LAST_RESULTS
    f1 = np.ascontiguousarray(np.asarray(inputs["frag1_layer"], np.float32))
    f2 = np.ascontiguousarray(np.asarray(inputs["frag2_layer"], np.float32))
    cx = np.ascontiguousarray(np.asarray(inputs["complex_layer"], np.float32))
    z1 = np.asarray(inputs["frag1_z"])
    z2 = np.asarray(inputs["frag2_z"])
    zc = np.asarray(inputs["complex_z"])
    W0 = np.asarray(inputs["W0"], np.float32)
    b0 = np.asarray(inputs["b0"], np.float32)
    W1 = np.asarray(inputs["W1"], np.float32)
    b1 = np.asarray(inputs["b1"], np.float32)
    W2 = np.asarray(inputs["W2"], np.float32)
    b2 = np.asarray(inputs["b2"], np.float32)
    Wout = np.asarray(inputs["Wout"], np.float32)
    bout = np.asarray(inputs["bout"], np.float32)

    _st_dt, np_dt = _dtypes()

    x_all = np.concatenate([f1, f2, cx], axis=1)          # [B, Na, F]
    z_all = np.concatenate([z1, z2, zc], axis=1)          # [B, Na]
    Bn, Na, _ = x_all.shape
    assert Bn == B
    sgn = np.concatenate([
        np.full(f1.shape[1], -1.0, np.float32),
        np.full(f2.shape[1], -1.0, np.float32),
        np.full(cx.shape[1], 1.0, np.float32),
    ])

    xf = x_all.reshape(-1, F)
    zf = z_all.reshape(-1).astype(np.int64)
    bidx = np.repeat(np.arange(B), Na)
    sf = np.tile(sgn, B)

    order = np.argsort(zf, kind="stable")
    counts = np.bincount(zf, minlength=T)[:T]
    GRAN = NCORES * PTILE
    padded = -(-counts // GRAN) * GRAN
    k_t = (padded // GRAN).astype(int)
    n_core = int(padded.sum()) // NCORES
    ntt = n_core // PTILE

    # Per-core atom index lists; -1 marks padding (all pads land on core 7's
    # tail of each type chunk — compute is identical on every core).
    per_core = [[] for _ in range(NCORES)]
    pos = 0
    for t in range(T):
        ct, pt = int(counts[t]), int(padded[t])
        idx = order[pos:pos + ct]
        pos += ct
        if pt == 0:
            continue
        ip = np.full(pt, -1, np.int64)
        ip[:ct] = idx
        ip = ip.reshape(NCORES, pt // NCORES)
        for c in range(NCORES):
            per_core[c].append(ip[c])
    idx_cores = np.stack([np.concatenate(l) for l in per_core])  # [NC, n_core]

    valid = idx_cores >= 0
    safe = np.where(valid, idx_cores, 0)
    xg = xf[safe]
    xg[~valid] = 0.0
    xT = np.ascontiguousarray(xg.transpose(0, 2, 1)).astype(np_dt)  # [NC,F,n]

    S = np.zeros((NCORES, n_core, B), np.float32)
    rows = sf[safe] * valid
    bcols = bidx[safe]
    S[np.arange(NCORES)[:, None], np.arange(n_core)[None, :], bcols] = rows
    S_dev = np.ascontiguousarray(
        S.reshape(NCORES, ntt, PTILE, B).transpose(0, 2, 1, 3)
    ).reshape(NCORES, PTILE, ntt * B)

    bpack = np.zeros((PTILE, T * 5), np.float32)
    for t in range(T):
        bpack[:, t * 5 + 0] = b0[t, :128]
        bpack[:, t * 5 + 1] = b0[t, 128:]
        bpack[:, t * 5 + 2] = b1[t, :128]
        bpack[:, t * 5 + 3] = b1[t, 128:]
        bpack[:, t * 5 + 4] = b2[t, :128]

    bias_term = np.bincount(bidx, weights=(sf * bout[zf, 0]).astype(np.float64),
                            minlength=B)[:B]

    nc = _build(k_t, n_core)
    shared = {
        "W0": np.ascontiguousarray(W0.astype(np_dt)),
        "W1": np.ascontiguousarray(W1.astype(np_dt)),
        "W2": np.ascontiguousarray(W2.astype(np_dt)),
        "Wout": np.ascontiguousarray(Wout.astype(np_dt)),
        "bpack": bpack,
    }
    in_maps = []
    for c in range(NCORES):
        m = dict(shared)
        m["xT"] = xT[c]
        m["S"] = S_dev[c]
        in_maps.append(m)

    kw = {}
    if TRACE:
        kw = dict(trace=True, trace_cores=list(range(NCORES)))
    res = run_bass_kernel_spmd(nc, in_maps, core_ids=list(range(NCORES)), **kw)
    LAST_RESULTS = res

    parts = np.stack([res.results[c]["res"][0].astype(np.float64)
                      for c in range(NCORES)])
    out = parts.sum(axis=0) + bias_term
    return out.astype(np.float32)[:, None]


# revision 38
# speedup vs baseline: 1.4796x; 1.4796x over previous
"""Trainium2 Bass kernel for nn_AtomicConvScore (MoE-routing style).

Strategy (routed / expert-grouped, data-parallel over atoms):
  * Concatenate frag1/frag2/complex atoms into one list with a per-atom
    sign (+1 complex, -1 frags); the answer is
        out[b] = sum_n sign[n] * MLP_{z[n]}(x[n])   (+ bout correction)
  * Sort atoms by type on the host, pad each type group to a multiple of
    8*128 and give each core 1/8 of every type group -> every core runs
    the IDENTICAL instruction schedule (SPMD) on different data.
  * On device, activations stay feature-major ([feature, atom] layout) so
    no transposes are ever needed:
        h^T = relu(W^T_chunk @ x^T_chunk + b)
  * Per-atom energies come from one row-matmul per supertile
    (Wout^T @ h3 -> [1, N] row); rows are staged to DRAM and reloaded
    atom-major [128, tiles], then reduced against a host-built signed
    per-batch mask (S[n, b] = sign * (batch == b)) with tiny accumulating
    matmuls in PSUM.
  * Host sums the 8 per-core partials and adds the tiny bout correction.
"""

import os
import sys

sys.path.insert(0, "/opt/trn_rl_repo")

import numpy as np
import ml_dtypes

import concourse.bass as bass
import concourse.tile as tile
from concourse import bacc, mybir
from concourse.bass_utils import run_bass_kernel_spmd

# Problem constants (hardcoded per the self-contained-kernel contract).
B = 16
F = 256
H1, H2, H3 = 256, 256, 128
T = 5
NCORES = 8
PTILE = 128  # atoms per tile (partition dim)
SUPER = 4    # tiles per supertile -> moving dim N = 512
CHUNK_TILES = 16  # tiles per x/o chunk (2048 atoms, ~1MB fp32 DMAs)
N_WARMUP = 3      # PE warmup matmuls issued under the DMA preamble

# Matmul precision mode: "bf16" (1 cyc/row), "f32r" (1 cyc/row @ N>=256,
# ~tf32 accuracy), "f32" (4 cyc/row, exact).
MM_MODE = os.environ.get("KMODE", "f32r")
# Debug bisect: comma list of {nored,nowarm,nofinal} to disable pieces.
KDBG = set(filter(None, os.environ.get("KDBG", "").split(",")))
# Reduction implementation: "bmm" (per-tile matmuls into PSUM) or
# "ttr0" (DVE tensor_tensor_reduce with literal init).
KRED = os.environ.get("KRED", "bmm")

TRACE = False          # test.py sets this for profiling runs
LAST_RESULTS = None    # test.py reads exec_time_ns from here

_F32 = mybir.dt.float32

# CONSTW column layout: per type t (always 5 blocks, inactive left zero):
#   +0    w0 chunks (k,m) at (k*2+m)*128
#   +512  w1 chunks (k,m)
#   +1024 w2 chunks (k)
#   +1280 wo2 (2 cols: Wout column, zeros)
_TBLK = 1282
_WCOLS = T * _TBLK


def _dtypes():
    if MM_MODE == "bf16":
        return mybir.dt.bfloat16, ml_dtypes.bfloat16
    if MM_MODE == "f32r":
        return mybir.dt.float32r, np.float32
    assert MM_MODE == "f32"
    return mybir.dt.float32, np.float32


def _schedule(k_t):
    """supertile schedule [(t, ntiles)] and chunk grouping [n_supertiles]."""
    sched = []
    for t in range(T):
        left = int(k_t[t])
        while left > 0:
            nt = min(SUPER, left)
            sched.append((t, nt))
            left -= nt
    chunks = []
    cur, cur_tiles = 0, 0
    cap = 2 * SUPER  # smaller first chunk so compute starts early
    for _, nt in sched:
        if cur_tiles + nt > cap and cur > 0:
            chunks.append(cur)
            cur, cur_tiles = 0, 0
            cap = CHUNK_TILES
        cur += 1
        cur_tiles += nt
    if cur:
        chunks.append(cur)
    # keep the last chunk short: its reduction chain is the kernel tail
    if len(chunks) > 1 and chunks[-1] > 1:
        last = chunks.pop()
        chunks.extend([last - 1, 1])
    return sched, chunks


def _build(k_t, n_core):
    """Build the (SPMD-uniform) Bass program for one core."""
    st_dt, _np_dt = _dtypes()
    ntt = n_core // PTILE
    fcols = 26 if KRED == "bmm" else 26 + B * ntt
    relu = mybir.ActivationFunctionType.Relu
    sched, chunks = _schedule(k_t)
    n_chunks = len(chunks)

    nc = bacc.Bacc()
    xT_d = nc.dram_tensor("xT", [F, n_core], st_dt, kind="ExternalInput")
    CW_d = nc.dram_tensor("CONSTW", [PTILE, _WCOLS], st_dt, kind="ExternalInput")
    CF_d = nc.dram_tensor("CONSTF", [PTILE, fcols], _F32, kind="ExternalInput")
    if KRED == "bmm":
        S2_d = nc.dram_tensor("S2", [PTILE, ntt * B], st_dt,
                              kind="ExternalInput")
        out_shape = [1, B]
    else:
        out_shape = [B, 1]
    out_d = nc.dram_tensor("res", out_shape, _F32, kind="ExternalOutput")

    active = sorted({t for t, _ in sched})
    total_tiles = sum(nt for _, nt in sched)

    with tile.TileContext(nc) as tc:
        with (
            tc.tile_pool(name="const", bufs=1) as cpool,
            tc.tile_pool(name="x", bufs=3) as xpool,
            tc.tile_pool(name="h", bufs=2) as hpool,
            tc.tile_pool(name="oc", bufs=2) as ocpool,
            tc.tile_pool(name="ors", bufs=2) as orspool,
            tc.tile_pool(name="dram", bufs=2, space="DRAM") as dpool,
            tc.tile_pool(name="pl1", bufs=3, space="PSUM") as pl1,
            tc.tile_pool(name="pl2", bufs=2, space="PSUM") as pl2,
            tc.tile_pool(name="pl3", bufs=1, space="PSUM") as pl3,
            tc.tile_pool(name="po", bufs=1, space="PSUM") as popool,
            tc.tile_pool(name="pres", bufs=1, space="PSUM") as prespool,
        ):
            # ---- PE warmup: keep the PE busy (and HAM warm) while the
            # constant/x DMAs stream in. Plain fp32 (4 cyc/row) so each
            # matmul covers ~1.7us of DMA time; values never read.
            if "nowarm" not in KDBG:
                wscr = cpool.tile([128, 512], _F32, tag="warm")
                nc.vector.memset(wscr[:], 0.0)
                wps = pl3.tile([128, 512], _F32, tag="pl3")
                for _ in range(N_WARMUP):
                    nc.tensor.matmul(wps[:], wscr[:, 0:128], wscr[:],
                                     start=True, stop=True)

            # ---- first x chunk + constants (issue order matters: the sync
            # queue drains in order, and compute waits on x0 + type-0
            # weights; S2 is not needed until the first chunk reduction).
            c0tiles = sum(nt for _, nt in sched[:chunks[0]])
            x0c0 = xpool.tile([128, c0tiles * PTILE], st_dt, tag="x0")
            nc.sync.dma_start(x0c0[:], xT_d[0:128, 0:c0tiles * PTILE])
            x1c0 = xpool.tile([128, c0tiles * PTILE], st_dt, tag="x1")
            nc.sync.dma_start(x1c0[:], xT_d[128:256, 0:c0tiles * PTILE])

            CW = cpool.tile([PTILE, _WCOLS], st_dt, tag="CW")
            t0 = active[0]
            nc.sync.dma_start(CW[:, t0 * _TBLK:(t0 + 1) * _TBLK],
                              CW_d[:, t0 * _TBLK:(t0 + 1) * _TBLK])
            CF = cpool.tile([PTILE, fcols], _F32, tag="CF")
            nc.sync.dma_start(CF[:], CF_d[:])

            # chunk 1's x goes next — the PE needs it before the remaining
            # type blocks (types are consumed in order).
            xc1 = None
            if len(chunks) > 1:
                c1tiles = sum(nt for _, nt in
                              sched[chunks[0]:chunks[0] + chunks[1]])
                c1off = c0tiles * PTILE
                x0c1 = xpool.tile([128, c1tiles * PTILE], st_dt, tag="x0")
                nc.sync.dma_start(x0c1[:],
                                  xT_d[0:128, c1off:c1off + c1tiles * PTILE])
                x1c1 = xpool.tile([128, c1tiles * PTILE], st_dt, tag="x1")
                nc.sync.dma_start(x1c1[:],
                                  xT_d[128:256, c1off:c1off + c1tiles * PTILE])
                xc1 = (x0c1, x1c1)

            for t in active[1:]:
                nc.sync.dma_start(CW[:, t * _TBLK:(t + 1) * _TBLK],
                                  CW_d[:, t * _TBLK:(t + 1) * _TBLK])
            if KRED == "bmm":
                S2 = cpool.tile([PTILE, ntt * B], st_dt, tag="S2")
                nc.sync.dma_start(S2[:], S2_d[:])

            def w0(t, k, m):
                c = t * _TBLK + (k * 2 + m) * 128
                return CW[:, c:c + 128]

            def w1(t, k, m):
                c = t * _TBLK + 512 + (k * 2 + m) * 128
                return CW[:, c:c + 128]

            def w2(t, k):
                c = t * _TBLK + 1024 + k * 128
                return CW[:, c:c + 128]

            def wo2(t):
                c = t * _TBLK + 1280
                return CW[:, c:c + 2]

            def bias(t, c):
                return CF[:, t * 5 + c:t * 5 + c + 1]

            ones_col = CF[:, 25:26]

            if KRED == "bmm":
                pres = prespool.tile([1, B], _F32, tag="pres")
            else:
                pcols = [cpool.tile([128, B], _F32, tag=f"pcols{i}",
                                    name=f"pcols{i}") for i in range(2)]
                junk = cpool.tile([128, CHUNK_TILES], _F32, tag="junk")

            # ---- chunk reduction (emitted one chunk late so the o-gather
            # DMA latency hides under the next chunk's compute) ----
            def reduce_chunk(o_chunk, jg0, ctiles):
                cN = ctiles * PTILE
                o_dram = dpool.tile([1, cN], st_dt, tag="odram",
                                    name="odram")
                nc.sync.dma_start(o_dram[:], o_chunk[:])
                o_rs = orspool.tile([128, ctiles], st_dt, tag="ors",
                                    name="ors")
                nc.sync.dma_start(
                    o_rs[:],
                    o_dram[0, :].rearrange("(p j) -> p j", p=128))
                if KRED == "bmm":
                    for j in range(ctiles):
                        jj = jg0 + j
                        nc.tensor.matmul(
                            pres[:], o_rs[:, j:j + 1],
                            S2[:, jj * B:(jj + 1) * B],
                            start=(jj == 0), stop=(jj == total_tiles - 1),
                            skip_group_check=True)
                else:
                    ci = reduce_chunk.ci
                    reduce_chunk.ci += 1
                    prev = pcols[(ci + 1) % 2]
                    cur = pcols[ci % 2]
                    for b in range(B):
                        c0 = 26 + b * ntt + jg0
                        nc.vector.tensor_tensor_reduce(
                            junk[:, 0:ctiles], o_rs[:],
                            CF[:, c0:c0 + ctiles],
                            1.0, 0.0,
                            mybir.AluOpType.mult, mybir.AluOpType.add,
                            cur[:, b:b + 1])
                    if ci > 0:
                        nc.vector.tensor_tensor(
                            cur[:], cur[:], prev[:], mybir.AluOpType.add)

            reduce_chunk.ci = 0

            # ---- main loop over chunks of supertiles ----
            si = 0       # supertile index
            col = 0      # atom column offset
            jg = 0       # global tile index
            pending = None
            for ci, n_super in enumerate(chunks):
                csts = sched[si:si + n_super]
                si += n_super
                ctiles = sum(nt for _, nt in csts)
                cN = ctiles * PTILE
                if ci == 0:
                    x0, x1 = x0c0, x1c0
                elif ci == 1 and xc1 is not None:
                    x0, x1 = xc1
                else:
                    x0 = xpool.tile([128, cN], st_dt, tag="x0")
                    nc.sync.dma_start(x0[:], xT_d[0:128, col:col + cN])
                    x1 = xpool.tile([128, cN], st_dt, tag="x1")
                    nc.sync.dma_start(x1[:], xT_d[128:256, col:col + cN])
                o_chunk = ocpool.tile([1, cN], st_dt, tag="oc")
                # p-major view [1, j, p]: element (j, p) at o_chunk[0,
                # p*ctiles + j] so the later atom-major reload reads
                # contiguous per-partition rows.
                o_view = o_chunk[0:1, :].rearrange(
                    "o (p j) -> o p j", p=128).rearrange("o p j -> o j p")

                ccol = 0
                for sti, (t, nt) in enumerate(csts):
                    N = PTILE * nt
                    xs0 = x0[:, ccol:ccol + N]
                    xs1 = x1[:, ccol:ccol + N]

                    def layer(xa, xb, wf, bc, pool, tag):
                        outs = []
                        for m in range(2):
                            p = pool.tile([128, N], _F32, tag=tag)
                            nc.tensor.matmul(p[:], wf(t, 0, m), xa[:],
                                             start=True, stop=False)
                            nc.tensor.matmul(p[:], wf(t, 1, m), xb[:],
                                             start=False, stop=True)
                            h = hpool.tile([128, N], st_dt, tag=f"{tag}_{m}")
                            if m == 0:
                                nc.scalar.activation(h[:], p[:], relu,
                                                     bias=bias(t, bc + m))
                            else:
                                nc.vector.tensor_scalar(
                                    h[:], p[:], bias(t, bc + m), 0.0,
                                    mybir.AluOpType.add, mybir.AluOpType.max)
                            outs.append(h)
                        return outs

                    h1 = layer(xs0, xs1, w0, 0, pl1, "ph1")
                    h2 = layer(h1[0], h1[1], w1, 2, pl2, "ph2")

                    p3 = pl3.tile([128, N], _F32, tag="pl3")
                    nc.tensor.matmul(p3[:], w2(t, 0), h2[0][:],
                                     start=True, stop=False)
                    nc.tensor.matmul(p3[:], w2(t, 1), h2[1][:],
                                     start=False, stop=True)
                    h3 = hpool.tile([128, N], st_dt, tag="h3")
                    if sti % 2 == 0:
                        nc.scalar.activation(h3[:], p3[:], relu,
                                             bias=bias(t, 4))
                    else:
                        nc.vector.tensor_scalar(
                            h3[:], p3[:], bias(t, 4), 0.0,
                            mybir.AluOpType.add, mybir.AluOpType.max)

                    # per-atom head: [1, N] energy row (row 1 is zeros)
                    po = popool.tile([2, N], _F32, tag="po")
                    nc.tensor.matmul(po[:], wo2(t), h3[:],
                                     start=True, stop=True)
                    lj = ccol // PTILE
                    nc.scalar.copy(
                        o_view[:, lj:lj + nt, :],
                        po[0:1, :].rearrange("o (j p) -> o j p", p=128))
                    ccol += N

                if "nored" not in KDBG:
                    if pending is not None:
                        reduce_chunk(*pending)
                    pending = (o_chunk, jg, ctiles)
                jg += ctiles
                col += cN
            if pending is not None:
                reduce_chunk(*pending)

            # ---- final ----
            if KRED == "bmm":
                res_sb = cpool.tile([1, B], _F32, tag="res")
                if "nored" in KDBG:
                    nc.vector.memset(res_sb[:], 0.0)
                else:
                    nc.scalar.copy(res_sb[:], pres[:])
            else:
                res_sb = cpool.tile([B, 1], _F32, tag="res")
                if KDBG & {"nored", "nofinal"}:
                    nc.vector.memset(res_sb[:], 0.0)
                else:
                    presf = prespool.tile([B, 1], _F32, tag="pres")
                    final = pcols[(n_chunks - 1) % 2]
                    nc.tensor.matmul(presf[:], final[:], ones_col,
                                     start=True, stop=True)
                    nc.scalar.copy(res_sb[:], presf[:])
            nc.sync.dma_start(out_d[:], res_sb[:])
    nc.finalize()
    return nc


def kernel(**inputs):
    global LAST_RESULTS
    f1 = np.ascontiguousarray(np.asarray(inputs["frag1_layer"], np.float32))
    f2 = np.ascontiguousarray(np.asarray(inputs["frag2_layer"], np.float32))
    cx = np.ascontiguousarray(np.asarray(inputs["complex_layer"], np.float32))
    z1 = np.asarray(inputs["frag1_z"])
    z2 = np.asarray(inputs["frag2_z"])
    zc = np.asarray(inputs["complex_z"])
    W0 = np.asarray(inputs["W0"], np.float32)
    b0 = np.asarray(inputs["b0"], np.float32)
    W1 = np.asarray(inputs["W1"], np.float32)
    b1 = np.asarray(inputs["b1"], np.float32)
    W2 = np.asarray(inputs["W2"], np.float32)
    b2 = np.asarray(inputs["b2"], np.float32)
    Wout = np.asarray(inputs["Wout"], np.float32)
    bout = np.asarray(inputs["bout"], np.float32)

    _st_dt, np_dt = _dtypes()

    x_all = np.concatenate([f1, f2, cx], axis=1)          # [B, Na, F]
    z_all = np.concatenate([z1, z2, zc], axis=1)          # [B, Na]
    Bn, Na, _ = x_all.shape
    assert Bn == B
    sgn = np.concatenate([
        np.full(f1.shape[1], -1.0, np.float32),
        np.full(f2.shape[1], -1.0, np.float32),
        np.full(cx.shape[1], 1.0, np.float32),
    ])

    xf = x_all.reshape(-1, F)
    zf = z_all.reshape(-1).astype(np.int64)
    bidx = np.repeat(np.arange(B), Na)
    sf = np.tile(sgn, B)

    order = np.argsort(zf, kind="stable")
    counts = np.bincount(zf, minlength=T)[:T]
    GRAN = NCORES * PTILE
    padded = -(-counts // GRAN) * GRAN
    k_t = (padded // GRAN).astype(int)
    n_core = int(padded.sum()) // NCORES
    ntt = n_core // PTILE

    # Per-core atom index lists; -1 marks padding (pads land on core 7's
    # tail of each type chunk — compute is identical on every core).
    per_core = [[] for _ in range(NCORES)]
    pos = 0
    for t in range(T):
        ct, pt = int(counts[t]), int(padded[t])
        idx = order[pos:pos + ct]
        pos += ct
        if pt == 0:
            continue
        ip = np.full(pt, -1, np.int64)
        ip[:ct] = idx
        ip = ip.reshape(NCORES, pt // NCORES)
        for c in range(NCORES):
            per_core[c].append(ip[c])
    idx_cores = np.stack([np.concatenate(l) for l in per_core])  # [NC, n]

    valid = idx_cores >= 0
    safe = np.where(valid, idx_cores, 0)
    xg = xf[safe]
    xg[~valid] = 0.0
    xT = np.ascontiguousarray(xg.transpose(0, 2, 1)).astype(np_dt)  # [NC,F,n]

    # S[c, n, b] = sign * (batch == b)
    S = np.zeros((NCORES, n_core, B), np.float32)
    rows = sf[safe] * valid
    bcols = bidx[safe]
    S[np.arange(NCORES)[:, None], np.arange(n_core)[None, :], bcols] = rows

    # CONSTW: weights packed per type in the _TBLK layout
    CWh = np.zeros((PTILE, _WCOLS), np.float32)
    for t in range(T):
        base = t * _TBLK
        for k in range(2):
            for m in range(2):
                CWh[:, base + (k * 2 + m) * 128:base + (k * 2 + m + 1) * 128] = \
                    W0[t, 128 * k:128 * (k + 1), 128 * m:128 * (m + 1)]
                CWh[:, base + 512 + (k * 2 + m) * 128:
                    base + 512 + (k * 2 + m + 1) * 128] = \
                    W1[t, 128 * k:128 * (k + 1), 128 * m:128 * (m + 1)]
            CWh[:, base + 1024 + k * 128:base + 1024 + (k + 1) * 128] = \
                W2[t, 128 * k:128 * (k + 1), 0:128]
        CWh[:, base + 1280] = Wout[t, :, 0]
    CWh = np.ascontiguousarray(CWh).astype(np_dt)

    # CONSTF: 25 bias cols + ones col (+ b-major S2 in ttr0 mode)
    CFh = np.zeros((PTILE, 26), np.float32)
    for t in range(T):
        CFh[:, t * 5 + 0] = b0[t, :128]
        CFh[:, t * 5 + 1] = b0[t, 128:]
        CFh[:, t * 5 + 2] = b1[t, :128]
        CFh[:, t * 5 + 3] = b1[t, 128:]
        CFh[:, t * 5 + 4] = b2[t, :128]
    CFh[:, 25] = 1.0

    bias_term = np.bincount(bidx, weights=(sf * bout[zf, 0]).astype(np.float64),
                            minlength=B)[:B]

    nc = _build(k_t, n_core)
    in_maps = []
    for c in range(NCORES):
        m = {"xT": xT[c], "CONSTW": CWh}
        if KRED == "bmm":
            m["CONSTF"] = CFh
            # S2[p, j*B + b] (tile-major)
            m["S2"] = np.ascontiguousarray(
                S[c].reshape(ntt, PTILE, B).transpose(1, 0, 2)
            ).reshape(PTILE, ntt * B).astype(np_dt)
        else:
            # b-major: S2[p, b*ntt + j]
            s2b = np.ascontiguousarray(
                S[c].reshape(ntt, PTILE, B).transpose(1, 2, 0)
            ).reshape(PTILE, B * ntt)
            m["CONSTF"] = np.ascontiguousarray(
                np.concatenate([CFh, s2b], axis=1))
        in_maps.append(m)

    kw = {}
    if TRACE:
        kw = dict(trace=True, trace_cores=list(range(NCORES)))
    res = run_bass_kernel_spmd(nc, in_maps, core_ids=list(range(NCORES)), **kw)
    LAST_RESULTS = res

    parts = np.stack([res.results[c]["res"].reshape(B).astype(np.float64)
                      for c in range(NCORES)])
    out = parts.sum(axis=0) + bias_term
    return out.astype(np.float32)[:, None]


# revision 40
# speedup vs baseline: 1.5257x; 1.0311x over previous
"""Trainium2 Bass kernel for nn_AtomicConvScore (MoE-routing style).

Strategy (routed / expert-grouped, data-parallel over atoms):
  * Concatenate frag1/frag2/complex atoms into one list with a per-atom
    sign (+1 complex, -1 frags); the answer is
        out[b] = sum_n sign[n] * MLP_{z[n]}(x[n])   (+ bout correction)
  * Sort atoms by type on the host, pad each type group to a multiple of
    8*128 and give each core 1/8 of every type group -> every core runs
    the IDENTICAL instruction schedule (SPMD) on different data.
  * On device, activations stay feature-major ([feature, atom] layout) so
    no transposes are ever needed:
        h^T = relu(W^T_chunk @ x^T_chunk + b)
  * Per-atom energies come from one row-matmul per supertile
    (Wout^T @ h3 -> [1, N] row); rows are staged to DRAM and reloaded
    atom-major [128, tiles], then reduced against a host-built signed
    per-batch mask (S[n, b] = sign * (batch == b)) with tiny accumulating
    matmuls in PSUM.
  * Host sums the 8 per-core partials and adds the tiny bout correction.
"""

import os
import sys

sys.path.insert(0, "/opt/trn_rl_repo")

import numpy as np
import ml_dtypes

import concourse.bass as bass
import concourse.tile as tile
from concourse import bacc, mybir
from concourse.bass_utils import run_bass_kernel_spmd

# Problem constants (hardcoded per the self-contained-kernel contract).
B = 16
F = 256
H1, H2, H3 = 256, 256, 128
T = 5
NCORES = 8
PTILE = 128  # atoms per tile (partition dim)
SUPER = 4    # tiles per supertile -> moving dim N = 512
CHUNK_TILES = 16  # tiles per x/o chunk (2048 atoms, ~1MB fp32 DMAs)
N_WARMUP = 7      # PE warmup matmuls issued under the DMA preamble

# Matmul precision mode: "bf16" (1 cyc/row), "f32r" (1 cyc/row @ N>=256,
# ~tf32 accuracy), "f32" (4 cyc/row, exact).
MM_MODE = os.environ.get("KMODE", "f32r")
# Debug bisect: comma list of {nored,nowarm,nofinal} to disable pieces.
KDBG = set(filter(None, os.environ.get("KDBG", "").split(",")))
# Reduction implementation: "bmm" (per-tile matmuls into PSUM) or
# "ttr0" (DVE tensor_tensor_reduce with literal init).
KRED = os.environ.get("KRED", "bmm")

TRACE = False          # test.py sets this for profiling runs
LAST_RESULTS = None    # test.py reads exec_time_ns from here

_F32 = mybir.dt.float32

# CONSTW column layout: per type t (always 5 blocks, inactive left zero):
#   +0    w0 chunks (k,m) at (k*2+m)*128
#   +512  w1 chunks (k,m)
#   +1024 w2 chunks (k)
#   +1280 wo2 (2 cols: Wout column, zeros)
_TBLK = 1282
_WCOLS = T * _TBLK


def _dtypes():
    if MM_MODE == "bf16":
        return mybir.dt.bfloat16, ml_dtypes.bfloat16
    if MM_MODE == "f32r":
        return mybir.dt.float32r, np.float32
    assert MM_MODE == "f32"
    return mybir.dt.float32, np.float32


def _schedule(k_t):
    """supertile schedule [(t, ntiles)] and chunk grouping [n_supertiles]."""
    sched = []
    for t in range(T):
        left = int(k_t[t])
        while left > 0:
            nt = min(SUPER, left)
            sched.append((t, nt))
            left -= nt
    chunks = []
    cur, cur_tiles = 0, 0
    cap = 2 * SUPER  # smaller first chunk so compute starts early
    for _, nt in sched:
        if cur_tiles + nt > cap and cur > 0:
            chunks.append(cur)
            cur, cur_tiles = 0, 0
            cap = CHUNK_TILES
        cur += 1
        cur_tiles += nt
    if cur:
        chunks.append(cur)
    # Taper the tail: each chunk's reduction (2-DMA chain + matmuls) is
    # emitted inside the NEXT chunk's body, so the final bodies must be
    # big enough to hide the preceding reduction but small enough that
    # the last flush is short. Split the final chunk into [rest, 2, 1].
    if len(chunks) > 1:
        last = chunks.pop()
        if last > 3:
            chunks.extend([last - 3, 2, 1])
        elif last == 3:
            chunks.extend([2, 1])
        elif last == 2:
            chunks.extend([1, 1])
        else:
            chunks.append(last)
    return sched, chunks


def _build(k_t, n_core):
    """Build the (SPMD-uniform) Bass program for one core."""
    st_dt, _np_dt = _dtypes()
    ntt = n_core // PTILE
    fcols = 26 if KRED == "bmm" else 26 + B * ntt
    relu = mybir.ActivationFunctionType.Relu
    sched, chunks = _schedule(k_t)
    n_chunks = len(chunks)

    nc = bacc.Bacc()
    xT_d = nc.dram_tensor("xT", [F, n_core], st_dt, kind="ExternalInput")
    CW_d = nc.dram_tensor("CONSTW", [PTILE, _WCOLS], st_dt, kind="ExternalInput")
    CF_d = nc.dram_tensor("CONSTF", [PTILE, fcols], _F32, kind="ExternalInput")
    if KRED == "bmm":
        S2_d = nc.dram_tensor("S2", [PTILE, ntt * B], st_dt,
                              kind="ExternalInput")
        out_shape = [1, B]
    else:
        out_shape = [B, 1]
    out_d = nc.dram_tensor("res", out_shape, _F32, kind="ExternalOutput")

    active = sorted({t for t, _ in sched})
    total_tiles = sum(nt for _, nt in sched)

    with tile.TileContext(nc) as tc:
        with (
            tc.tile_pool(name="const", bufs=1) as cpool,
            tc.tile_pool(name="x", bufs=3) as xpool,
            tc.tile_pool(name="h", bufs=2) as hpool,
            tc.tile_pool(name="oc", bufs=2) as ocpool,
            tc.tile_pool(name="ors", bufs=2) as orspool,
            tc.tile_pool(name="dram", bufs=2, space="DRAM") as dpool,
            tc.tile_pool(name="pl1", bufs=3, space="PSUM") as pl1,
            tc.tile_pool(name="pl2", bufs=2, space="PSUM") as pl2,
            tc.tile_pool(name="pl3", bufs=1, space="PSUM") as pl3,
            tc.tile_pool(name="po", bufs=1, space="PSUM") as popool,
            tc.tile_pool(name="pres", bufs=1, space="PSUM") as prespool,
        ):
            # ---- PE warmup: keep the PE busy (and HAM warm) while the
            # constant/x DMAs stream in. Plain fp32 (4 cyc/row) so each
            # matmul covers ~1.7us of DMA time; values never read.
            if "nowarm" not in KDBG:
                wscr = cpool.tile([128, 512], _F32, tag="warm")
                nc.vector.memset(wscr[:], 0.0)
                wps = pl3.tile([128, 512], _F32, tag="pl3")
                for _ in range(N_WARMUP):
                    nc.tensor.matmul(wps[:], wscr[:, 0:128], wscr[:],
                                     start=True, stop=True)

            # ---- first x chunk + constants (issue order matters: the sync
            # queue drains in order, and compute waits on x0 + type-0
            # weights; S2 is not needed until the first chunk reduction).
            c0tiles = sum(nt for _, nt in sched[:chunks[0]])
            x0c0 = xpool.tile([128, c0tiles * PTILE], st_dt, tag="x0")
            nc.sync.dma_start(x0c0[:], xT_d[0:128, 0:c0tiles * PTILE])
            x1c0 = xpool.tile([128, c0tiles * PTILE], st_dt, tag="x1")
            nc.sync.dma_start(x1c0[:], xT_d[128:256, 0:c0tiles * PTILE])

            CW = cpool.tile([PTILE, _WCOLS], st_dt, tag="CW")
            t0 = active[0]
            nc.sync.dma_start(CW[:, t0 * _TBLK:(t0 + 1) * _TBLK],
                              CW_d[:, t0 * _TBLK:(t0 + 1) * _TBLK])
            CF = cpool.tile([PTILE, fcols], _F32, tag="CF")
            nc.sync.dma_start(CF[:], CF_d[:])

            # chunk 1's x goes next — the PE needs it before the remaining
            # type blocks (types are consumed in order).
            xc1 = None
            if len(chunks) > 1:
                c1tiles = sum(nt for _, nt in
                              sched[chunks[0]:chunks[0] + chunks[1]])
                c1off = c0tiles * PTILE
                x0c1 = xpool.tile([128, c1tiles * PTILE], st_dt, tag="x0")
                nc.sync.dma_start(x0c1[:],
                                  xT_d[0:128, c1off:c1off + c1tiles * PTILE])
                x1c1 = xpool.tile([128, c1tiles * PTILE], st_dt, tag="x1")
                nc.sync.dma_start(x1c1[:],
                                  xT_d[128:256, c1off:c1off + c1tiles * PTILE])
                xc1 = (x0c1, x1c1)

            for t in active[1:]:
                nc.sync.dma_start(CW[:, t * _TBLK:(t + 1) * _TBLK],
                                  CW_d[:, t * _TBLK:(t + 1) * _TBLK])
            if KRED == "bmm":
                S2 = cpool.tile([PTILE, ntt * B], st_dt, tag="S2")
                nc.sync.dma_start(S2[:], S2_d[:])

            def w0(t, k, m):
                c = t * _TBLK + (k * 2 + m) * 128
                return CW[:, c:c + 128]

            def w1(t, k, m):
                c = t * _TBLK + 512 + (k * 2 + m) * 128
                return CW[:, c:c + 128]

            def w2(t, k):
                c = t * _TBLK + 1024 + k * 128
                return CW[:, c:c + 128]

            def wo2(t):
                c = t * _TBLK + 1280
                return CW[:, c:c + 2]

            def bias(t, c):
                return CF[:, t * 5 + c:t * 5 + c + 1]

            ones_col = CF[:, 25:26]

            if KRED == "bmm":
                pres = prespool.tile([1, B], _F32, tag="pres")
            else:
                pcols = [cpool.tile([128, B], _F32, tag=f"pcols{i}",
                                    name=f"pcols{i}") for i in range(2)]
                junk = cpool.tile([128, CHUNK_TILES], _F32, tag="junk")

            # ---- chunk reduction (emitted one chunk late so the o-gather
            # DMA latency hides under the next chunk's compute) ----
            def reduce_chunk(o_chunk, jg0, ctiles):
                cN = ctiles * PTILE
                o_dram = dpool.tile([1, cN], st_dt, tag="odram",
                                    name="odram")
                nc.sync.dma_start(o_dram[:], o_chunk[:])
                o_rs = orspool.tile([128, ctiles], st_dt, tag="ors",
                                    name="ors")
                nc.sync.dma_start(
                    o_rs[:],
                    o_dram[0, :].rearrange("(p j) -> p j", p=128))
                if KRED == "bmm":
                    for j in range(ctiles):
                        jj = jg0 + j
                        nc.tensor.matmul(
                            pres[:], o_rs[:, j:j + 1],
                            S2[:, jj * B:(jj + 1) * B],
                            start=(jj == 0), stop=(jj == total_tiles - 1),
                            skip_group_check=True)
                else:
                    ci = reduce_chunk.ci
                    reduce_chunk.ci += 1
                    prev = pcols[(ci + 1) % 2]
                    cur = pcols[ci % 2]
                    for b in range(B):
                        c0 = 26 + b * ntt + jg0
                        nc.vector.tensor_tensor_reduce(
                            junk[:, 0:ctiles], o_rs[:],
                            CF[:, c0:c0 + ctiles],
                            1.0, 0.0,
                            mybir.AluOpType.mult, mybir.AluOpType.add,
                            cur[:, b:b + 1])
                    if ci > 0:
                        nc.vector.tensor_tensor(
                            cur[:], cur[:], prev[:], mybir.AluOpType.add)

            reduce_chunk.ci = 0

            # ---- main loop over chunks of supertiles ----
            si = 0       # supertile index
            col = 0      # atom column offset
            jg = 0       # global tile index
            pending = None
            for ci, n_super in enumerate(chunks):
                csts = sched[si:si + n_super]
                si += n_super
                ctiles = sum(nt for _, nt in csts)
                cN = ctiles * PTILE
                if ci == 0:
                    x0, x1 = x0c0, x1c0
                elif ci == 1 and xc1 is not None:
                    x0, x1 = xc1
                else:
                    x0 = xpool.tile([128, cN], st_dt, tag="x0")
                    nc.sync.dma_start(x0[:], xT_d[0:128, col:col + cN])
                    x1 = xpool.tile([128, cN], st_dt, tag="x1")
                    nc.sync.dma_start(x1[:], xT_d[128:256, col:col + cN])
                o_chunk = ocpool.tile([1, cN], st_dt, tag="oc")
                # p-major view [1, j, p]: element (j, p) at o_chunk[0,
                # p*ctiles + j] so the later atom-major reload reads
                # contiguous per-partition rows.
                o_view = o_chunk[0:1, :].rearrange(
                    "o (p j) -> o p j", p=128).rearrange("o p j -> o j p")

                ccol = 0
                for sti, (t, nt) in enumerate(csts):
                    N = PTILE * nt
                    xs0 = x0[:, ccol:ccol + N]
                    xs1 = x1[:, ccol:ccol + N]

                    def layer(xa, xb, wf, bc, pool, tag):
                        outs = []
                        for m in range(2):
                            p = pool.tile([128, N], _F32, tag=tag)
                            nc.tensor.matmul(p[:], wf(t, 0, m), xa[:],
                                             start=True, stop=False)
                            nc.tensor.matmul(p[:], wf(t, 1, m), xb[:],
                                             start=False, stop=True)
                            h = hpool.tile([128, N], st_dt, tag=f"{tag}_{m}")
                            if m == 0:
                                nc.scalar.activation(h[:], p[:], relu,
                                                     bias=bias(t, bc + m))
                            else:
                                nc.vector.tensor_scalar(
                                    h[:], p[:], bias(t, bc + m), 0.0,
                                    mybir.AluOpType.add, mybir.AluOpType.max)
                            outs.append(h)
                        return outs

                    h1 = layer(xs0, xs1, w0, 0, pl1, "ph1")
                    h2 = layer(h1[0], h1[1], w1, 2, pl2, "ph2")

                    p3 = pl3.tile([128, N], _F32, tag="pl3")
                    nc.tensor.matmul(p3[:], w2(t, 0), h2[0][:],
                                     start=True, stop=False)
                    nc.tensor.matmul(p3[:], w2(t, 1), h2[1][:],
                                     start=False, stop=True)
                    h3 = hpool.tile([128, N], st_dt, tag="h3")
                    if sti % 2 == 0:
                        nc.scalar.activation(h3[:], p3[:], relu,
                                             bias=bias(t, 4))
                    else:
                        nc.vector.tensor_scalar(
                            h3[:], p3[:], bias(t, 4), 0.0,
                            mybir.AluOpType.add, mybir.AluOpType.max)

                    # per-atom head: [1, N] energy row (row 1 is zeros)
                    po = popool.tile([2, N], _F32, tag="po")
                    nc.tensor.matmul(po[:], wo2(t), h3[:],
                                     start=True, stop=True)
                    lj = ccol // PTILE
                    nc.scalar.copy(
                        o_view[:, lj:lj + nt, :],
                        po[0:1, :].rearrange("o (j p) -> o j p", p=128))
                    ccol += N

                if "nored" not in KDBG:
                    if pending is not None:
                        reduce_chunk(*pending)
                    pending = (o_chunk, jg, ctiles)
                jg += ctiles
                col += cN
            if pending is not None:
                reduce_chunk(*pending)

            # ---- final ----
            if KRED == "bmm":
                res_sb = cpool.tile([1, B], _F32, tag="res")
                if "nored" in KDBG:
                    nc.vector.memset(res_sb[:], 0.0)
                else:
                    nc.scalar.copy(res_sb[:], pres[:])
            else:
                res_sb = cpool.tile([B, 1], _F32, tag="res")
                if KDBG & {"nored", "nofinal"}:
                    nc.vector.memset(res_sb[:], 0.0)
                else:
                    presf = prespool.tile([B, 1], _F32, tag="pres")
                    final = pcols[(n_chunks - 1) % 2]
                    nc.tensor.matmul(presf[:], final[:], ones_col,
                                     start=True, stop=True)
                    nc.scalar.copy(res_sb[:], presf[:])
            nc.sync.dma_start(out_d[:], res_sb[:])
    nc.finalize()
    return nc


def kernel(**inputs):
    global LAST_RESULTS
    f1 = np.ascontiguousarray(np.asarray(inputs["frag1_layer"], np.float32))
    f2 = np.ascontiguousarray(np.asarray(inputs["frag2_layer"], np.float32))
    cx = np.ascontiguousarray(np.asarray(inputs["complex_layer"], np.float32))
    z1 = np.asarray(inputs["frag1_z"])
    z2 = np.asarray(inputs["frag2_z"])
    zc = np.asarray(inputs["complex_z"])
    W0 = np.asarray(inputs["W0"], np.float32)
    b0 = np.asarray(inputs["b0"], np.float32)
    W1 = np.asarray(inputs["W1"], np.float32)
    b1 = np.asarray(inputs["b1"], np.float32)
    W2 = np.asarray(inputs["W2"], np.float32)
    b2 = np.asarray(inputs["b2"], np.float32)
    Wout = np.asarray(inputs["Wout"], np.float32)
    bout = np.asarray(inputs["bout"], np.float32)

    _st_dt, np_dt = _dtypes()

    x_all = np.concatenate([f1, f2, cx], axis=1)          # [B, Na, F]
    z_all = np.concatenate([z1, z2, zc], axis=1)          # [B, Na]
    Bn, Na, _ = x_all.shape
    assert Bn == B
    sgn = np.concatenate([
        np.full(f1.shape[1], -1.0, np.float32),
        np.full(f2.shape[1], -1.0, np.float32),
        np.full(cx.shape[1], 1.0, np.float32),
    ])

    xf = x_all.reshape(-1, F)
    zf = z_all.reshape(-1).astype(np.int64)
    bidx = np.repeat(np.arange(B), Na)
    sf = np.tile(sgn, B)

    order = np.argsort(zf, kind="stable")
    counts = np.bincount(zf, minlength=T)[:T]
    GRAN = NCORES * PTILE
    padded = -(-counts // GRAN) * GRAN
    k_t = (padded // GRAN).astype(int)
    n_core = int(padded.sum()) // NCORES
    ntt = n_core // PTILE

    # Per-core atom index lists; -1 marks padding (pads land on core 7's
    # tail of each type chunk — compute is identical on every core).
    per_core = [[] for _ in range(NCORES)]
    pos = 0
    for t in range(T):
        ct, pt = int(counts[t]), int(padded[t])
        idx = order[pos:pos + ct]
        pos += ct
        if pt == 0:
            continue
        ip = np.full(pt, -1, np.int64)
        ip[:ct] = idx
        ip = ip.reshape(NCORES, pt // NCORES)
        for c in range(NCORES):
            per_core[c].append(ip[c])
    idx_cores = np.stack([np.concatenate(l) for l in per_core])  # [NC, n]

    valid = idx_cores >= 0
    safe = np.where(valid, idx_cores, 0)
    xg = xf[safe]
    xg[~valid] = 0.0
    xT = np.ascontiguousarray(xg.transpose(0, 2, 1)).astype(np_dt)  # [NC,F,n]

    # S[c, n, b] = sign * (batch == b)
    S = np.zeros((NCORES, n_core, B), np.float32)
    rows = sf[safe] * valid
    bcols = bidx[safe]
    S[np.arange(NCORES)[:, None], np.arange(n_core)[None, :], bcols] = rows

    # CONSTW: weights packed per type in the _TBLK layout
    CWh = np.zeros((PTILE, _WCOLS), np.float32)
    for t in range(T):
        base = t * _TBLK
        for k in range(2):
            for m in range(2):
                CWh[:, base + (k * 2 + m) * 128:base + (k * 2 + m + 1) * 128] = \
                    W0[t, 128 * k:128 * (k + 1), 128 * m:128 * (m + 1)]
                CWh[:, base + 512 + (k * 2 + m) * 128:
                    base + 512 + (k * 2 + m + 1) * 128] = \
                    W1[t, 128 * k:128 * (k + 1), 128 * m:128 * (m + 1)]
            CWh[:, base + 1024 + k * 128:base + 1024 + (k + 1) * 128] = \
                W2[t, 128 * k:128 * (k + 1), 0:128]
        CWh[:, base + 1280] = Wout[t, :, 0]
    CWh = np.ascontiguousarray(CWh).astype(np_dt)

    # CONSTF: 25 bias cols + ones col (+ b-major S2 in ttr0 mode)
    CFh = np.zeros((PTILE, 26), np.float32)
    for t in range(T):
        CFh[:, t * 5 + 0] = b0[t, :128]
        CFh[:, t * 5 + 1] = b0[t, 128:]
        CFh[:, t * 5 + 2] = b1[t, :128]
        CFh[:, t * 5 + 3] = b1[t, 128:]
        CFh[:, t * 5 + 4] = b2[t, :128]
    CFh[:, 25] = 1.0

    bias_term = np.bincount(bidx, weights=(sf * bout[zf, 0]).astype(np.float64),
                            minlength=B)[:B]

    nc = _build(k_t, n_core)
    in_maps = []
    for c in range(NCORES):
        m = {"xT": xT[c], "CONSTW": CWh}
        if KRED == "bmm":
            m["CONSTF"] = CFh
            # S2[p, j*B + b] (tile-major)
            m["S2"] = np.ascontiguousarray(
                S[c].reshape(ntt, PTILE, B).transpose(1, 0, 2)
            ).reshape(PTILE, ntt * B).astype(np_dt)
        else:
            # b-major: S2[p, b*ntt + j]
            s2b = np.ascontiguousarray(
                S[c].reshape(ntt, PTILE, B).transpose(1, 2, 0)
            ).reshape(PTILE, B * ntt)
            m["CONSTF"] = np.ascontiguousarray(
                np.concatenate([CFh, s2b], axis=1))
        in_maps.append(m)

    kw = {}
    if TRACE:
        kw = dict(trace=True, trace_cores=list(range(NCORES)))
    res = run_bass_kernel_spmd(nc, in_maps, core_ids=list(range(NCORES)), **kw)
    LAST_RESULTS = res

    parts = np.stack([res.results[c]["res"].reshape(B).astype(np.float64)
                      for c in range(NCORES)])
    out = parts.sum(axis=0) + bias_term
    return out.astype(np.float32)[:, None]


# revision 41
# speedup vs baseline: 1.5529x; 1.0178x over previous
"""Trainium2 Bass kernel for nn_AtomicConvScore (MoE-routing style).

Strategy (routed / expert-grouped, data-parallel over atoms):
  * Concatenate frag1/frag2/complex atoms into one list with a per-atom
    sign (+1 complex, -1 frags); the answer is
        out[b] = sum_n sign[n] * MLP_{z[n]}(x[n])   (+ bout correction)
  * Sort atoms by type on the host, pad each type group to a multiple of
    8*128 and give each core 1/8 of every type group -> every core runs
    the IDENTICAL instruction schedule (SPMD) on different data.
  * On device, activations stay feature-major ([feature, atom] layout) so
    no transposes are ever needed:
        h^T = relu(W^T_chunk @ x^T_chunk + b)
  * Per-atom energies come from one row-matmul per supertile
    (Wout^T @ h3 -> [1, N] row); rows are staged to DRAM and reloaded
    atom-major [128, tiles], then reduced against a host-built signed
    per-batch mask (S[n, b] = sign * (batch == b)) with tiny accumulating
    matmuls in PSUM.
  * Host sums the 8 per-core partials and adds the tiny bout correction.
"""

import os
import sys

sys.path.insert(0, "/opt/trn_rl_repo")

import numpy as np
import ml_dtypes

import concourse.bass as bass
import concourse.tile as tile
from concourse import bacc, mybir
from concourse.bass_utils import run_bass_kernel_spmd

# Problem constants (hardcoded per the self-contained-kernel contract).
B = 16
F = 256
H1, H2, H3 = 256, 256, 128
T = 5
NCORES = 8
PTILE = 128  # atoms per tile (partition dim)
SUPER = 4    # tiles per supertile -> moving dim N = 512
CHUNK_TILES = 16  # tiles per x/o chunk (2048 atoms, ~1MB fp32 DMAs)
N_WARMUP = 7      # PE warmup matmuls issued under the DMA preamble

# Matmul precision mode: "bf16" (1 cyc/row), "f32r" (1 cyc/row @ N>=256,
# ~tf32 accuracy), "f32" (4 cyc/row, exact).
MM_MODE = os.environ.get("KMODE", "f32r")
# Debug bisect: comma list of {nored,nowarm,nofinal} to disable pieces.
KDBG = set(filter(None, os.environ.get("KDBG", "").split(",")))
# Reduction implementation: "bmm" (per-tile matmuls into PSUM) or
# "ttr0" (DVE tensor_tensor_reduce with literal init).
KRED = os.environ.get("KRED", "bmm")

TRACE = False          # test.py sets this for profiling runs
LAST_RESULTS = None    # test.py reads exec_time_ns from here

_F32 = mybir.dt.float32

# CONSTW column layout: per type t (always 5 blocks, inactive left zero):
#   +0    w0 chunks (k,m) at (k*2+m)*128
#   +512  w1 chunks (k,m)
#   +1024 w2 chunks (k)
#   +1280 wo2 (2 cols: Wout column, zeros)
_TBLK = 1282
_WCOLS = T * _TBLK


def _dtypes():
    if MM_MODE == "bf16":
        return mybir.dt.bfloat16, ml_dtypes.bfloat16
    if MM_MODE == "f32r":
        return mybir.dt.float32r, np.float32
    assert MM_MODE == "f32"
    return mybir.dt.float32, np.float32


def _schedule(k_t):
    """supertile schedule [(t, ntiles)] and chunk grouping [n_supertiles]."""
    sched = []
    for t in range(T):
        left = int(k_t[t])
        while left > 0:
            nt = min(SUPER, left)
            sched.append((t, nt))
            left -= nt
    chunks = []
    cur, cur_tiles = 0, 0
    cap = 2 * SUPER  # smaller first chunk so compute starts early
    for _, nt in sched:
        if cur_tiles + nt > cap and cur > 0:
            chunks.append(cur)
            cur, cur_tiles = 0, 0
            cap = CHUNK_TILES
        cur += 1
        cur_tiles += nt
    if cur:
        chunks.append(cur)
    # Taper the tail: each chunk's reduction (2-DMA chain + matmuls) is
    # emitted inside the NEXT chunk's body, so the final bodies must be
    # big enough to hide the preceding reduction but small enough that
    # the last flush is short. Rebuild the tail as [rest, 2, 1].
    if len(chunks) > 2:
        l2 = chunks.pop() + chunks.pop()
        if l2 > 3:
            chunks.extend([l2 - 3, 2, 1])
        elif l2 == 3:
            chunks.extend([2, 1])
        else:
            chunks.extend([1] * l2)
    return sched, chunks


def _build(k_t, n_core):
    """Build the (SPMD-uniform) Bass program for one core."""
    st_dt, _np_dt = _dtypes()
    ntt = n_core // PTILE
    fcols = 26 if KRED == "bmm" else 26 + B * ntt
    relu = mybir.ActivationFunctionType.Relu
    sched, chunks = _schedule(k_t)
    n_chunks = len(chunks)

    nc = bacc.Bacc()
    xT_d = nc.dram_tensor("xT", [F, n_core], st_dt, kind="ExternalInput")
    CW_d = nc.dram_tensor("CONSTW", [PTILE, _WCOLS], st_dt, kind="ExternalInput")
    CF_d = nc.dram_tensor("CONSTF", [PTILE, fcols], _F32, kind="ExternalInput")
    if KRED == "bmm":
        S2_d = nc.dram_tensor("S2", [PTILE, ntt * B], st_dt,
                              kind="ExternalInput")
        out_shape = [1, B]
    else:
        out_shape = [B, 1]
    out_d = nc.dram_tensor("res", out_shape, _F32, kind="ExternalOutput")

    active = sorted({t for t, _ in sched})
    total_tiles = sum(nt for _, nt in sched)

    with tile.TileContext(nc) as tc:
        with (
            tc.tile_pool(name="const", bufs=1) as cpool,
            tc.tile_pool(name="x", bufs=3) as xpool,
            tc.tile_pool(name="h", bufs=2) as hpool,
            tc.tile_pool(name="oc", bufs=2) as ocpool,
            tc.tile_pool(name="ors", bufs=2) as orspool,
            tc.tile_pool(name="dram", bufs=2, space="DRAM") as dpool,
            tc.tile_pool(name="pl1", bufs=3, space="PSUM") as pl1,
            tc.tile_pool(name="pl2", bufs=2, space="PSUM") as pl2,
            tc.tile_pool(name="pl3", bufs=1, space="PSUM") as pl3,
            tc.tile_pool(name="po", bufs=1, space="PSUM") as popool,
            tc.tile_pool(name="pres", bufs=1, space="PSUM") as prespool,
        ):
            # ---- PE warmup: keep the PE busy (and HAM warm) while the
            # constant/x DMAs stream in. Plain fp32 (4 cyc/row) so each
            # matmul covers ~1.7us of DMA time; values never read.
            if "nowarm" not in KDBG:
                wscr = cpool.tile([128, 512], _F32, tag="warm")
                nc.vector.memset(wscr[:], 0.0)
                wps = pl3.tile([128, 512], _F32, tag="pl3")
                for _ in range(N_WARMUP):
                    nc.tensor.matmul(wps[:], wscr[:, 0:128], wscr[:],
                                     start=True, stop=True)

            # ---- first x chunk + constants (issue order matters: the sync
            # queue drains in order, and compute waits on x0 + type-0
            # weights; S2 is not needed until the first chunk reduction).
            c0tiles = sum(nt for _, nt in sched[:chunks[0]])
            x0c0 = xpool.tile([128, c0tiles * PTILE], st_dt, tag="x0")
            nc.sync.dma_start(x0c0[:], xT_d[0:128, 0:c0tiles * PTILE])
            x1c0 = xpool.tile([128, c0tiles * PTILE], st_dt, tag="x1")
            nc.sync.dma_start(x1c0[:], xT_d[128:256, 0:c0tiles * PTILE])

            CW = cpool.tile([PTILE, _WCOLS], st_dt, tag="CW")
            t0 = active[0]
            nc.sync.dma_start(CW[:, t0 * _TBLK:(t0 + 1) * _TBLK],
                              CW_d[:, t0 * _TBLK:(t0 + 1) * _TBLK])
            CF = cpool.tile([PTILE, fcols], _F32, tag="CF")
            nc.sync.dma_start(CF[:], CF_d[:])

            # chunk 1's x goes next — the PE needs it before the remaining
            # type blocks (types are consumed in order).
            xc1 = None
            if len(chunks) > 1:
                c1tiles = sum(nt for _, nt in
                              sched[chunks[0]:chunks[0] + chunks[1]])
                c1off = c0tiles * PTILE
                x0c1 = xpool.tile([128, c1tiles * PTILE], st_dt, tag="x0")
                nc.sync.dma_start(x0c1[:],
                                  xT_d[0:128, c1off:c1off + c1tiles * PTILE])
                x1c1 = xpool.tile([128, c1tiles * PTILE], st_dt, tag="x1")
                nc.sync.dma_start(x1c1[:],
                                  xT_d[128:256, c1off:c1off + c1tiles * PTILE])
                xc1 = (x0c1, x1c1)

            for t in active[1:]:
                nc.sync.dma_start(CW[:, t * _TBLK:(t + 1) * _TBLK],
                                  CW_d[:, t * _TBLK:(t + 1) * _TBLK])
            if KRED == "bmm":
                S2 = cpool.tile([PTILE, ntt * B], st_dt, tag="S2")
                nc.sync.dma_start(S2[:], S2_d[:])

            def w0(t, k, m):
                c = t * _TBLK + (k * 2 + m) * 128
                return CW[:, c:c + 128]

            def w1(t, k, m):
                c = t * _TBLK + 512 + (k * 2 + m) * 128
                return CW[:, c:c + 128]

            def w2(t, k):
                c = t * _TBLK + 1024 + k * 128
                return CW[:, c:c + 128]

            def wo2(t):
                c = t * _TBLK + 1280
                return CW[:, c:c + 2]

            def bias(t, c):
                return CF[:, t * 5 + c:t * 5 + c + 1]

            ones_col = CF[:, 25:26]

            if KRED == "bmm":
                pres = prespool.tile([1, B], _F32, tag="pres")
            else:
                pcols = [cpool.tile([128, B], _F32, tag=f"pcols{i}",
                                    name=f"pcols{i}") for i in range(2)]
                junk = cpool.tile([128, CHUNK_TILES], _F32, tag="junk")

            # ---- chunk reduction (emitted one chunk late so the o-gather
            # DMA latency hides under the next chunk's compute) ----
            def reduce_chunk(o_chunk, jg0, ctiles):
                cN = ctiles * PTILE
                o_dram = dpool.tile([1, cN], st_dt, tag="odram",
                                    name="odram")
                nc.sync.dma_start(o_dram[:], o_chunk[:])
                o_rs = orspool.tile([128, ctiles], st_dt, tag="ors",
                                    name="ors")
                nc.sync.dma_start(
                    o_rs[:],
                    o_dram[0, :].rearrange("(p j) -> p j", p=128))
                if KRED == "bmm":
                    for j in range(ctiles):
                        jj = jg0 + j
                        nc.tensor.matmul(
                            pres[:], o_rs[:, j:j + 1],
                            S2[:, jj * B:(jj + 1) * B],
                            start=(jj == 0), stop=(jj == total_tiles - 1),
                            skip_group_check=True)
                else:
                    ci = reduce_chunk.ci
                    reduce_chunk.ci += 1
                    prev = pcols[(ci + 1) % 2]
                    cur = pcols[ci % 2]
                    for b in range(B):
                        c0 = 26 + b * ntt + jg0
                        nc.vector.tensor_tensor_reduce(
                            junk[:, 0:ctiles], o_rs[:],
                            CF[:, c0:c0 + ctiles],
                            1.0, 0.0,
                            mybir.AluOpType.mult, mybir.AluOpType.add,
                            cur[:, b:b + 1])
                    if ci > 0:
                        nc.vector.tensor_tensor(
                            cur[:], cur[:], prev[:], mybir.AluOpType.add)

            reduce_chunk.ci = 0

            # ---- main loop over chunks of supertiles ----
            si = 0       # supertile index
            col = 0      # atom column offset
            jg = 0       # global tile index
            pending = None
            for ci, n_super in enumerate(chunks):
                csts = sched[si:si + n_super]
                si += n_super
                ctiles = sum(nt for _, nt in csts)
                cN = ctiles * PTILE
                if ci == 0:
                    x0, x1 = x0c0, x1c0
                elif ci == 1 and xc1 is not None:
                    x0, x1 = xc1
                else:
                    x0 = xpool.tile([128, cN], st_dt, tag="x0")
                    nc.sync.dma_start(x0[:], xT_d[0:128, col:col + cN])
                    x1 = xpool.tile([128, cN], st_dt, tag="x1")
                    nc.sync.dma_start(x1[:], xT_d[128:256, col:col + cN])
                o_chunk = ocpool.tile([1, cN], st_dt, tag="oc")
                # p-major view [1, j, p]: element (j, p) at o_chunk[0,
                # p*ctiles + j] so the later atom-major reload reads
                # contiguous per-partition rows.
                o_view = o_chunk[0:1, :].rearrange(
                    "o (p j) -> o p j", p=128).rearrange("o p j -> o j p")

                ccol = 0
                for sti, (t, nt) in enumerate(csts):
                    N = PTILE * nt
                    xs0 = x0[:, ccol:ccol + N]
                    xs1 = x1[:, ccol:ccol + N]

                    def layer(xa, xb, wf, bc, pool, tag):
                        outs = []
                        for m in range(2):
                            p = pool.tile([128, N], _F32, tag=tag)
                            nc.tensor.matmul(p[:], wf(t, 0, m), xa[:],
                                             start=True, stop=False)
                            nc.tensor.matmul(p[:], wf(t, 1, m), xb[:],
                                             start=False, stop=True)
                            h = hpool.tile([128, N], st_dt, tag=f"{tag}_{m}")
                            if m == 0:
                                nc.scalar.activation(h[:], p[:], relu,
                                                     bias=bias(t, bc + m))
                            else:
                                nc.vector.tensor_scalar(
                                    h[:], p[:], bias(t, bc + m), 0.0,
                                    mybir.AluOpType.add, mybir.AluOpType.max)
                            outs.append(h)
                        return outs

                    h1 = layer(xs0, xs1, w0, 0, pl1, "ph1")
                    h2 = layer(h1[0], h1[1], w1, 2, pl2, "ph2")

                    p3 = pl3.tile([128, N], _F32, tag="pl3")
                    nc.tensor.matmul(p3[:], w2(t, 0), h2[0][:],
                                     start=True, stop=False)
                    nc.tensor.matmul(p3[:], w2(t, 1), h2[1][:],
                                     start=False, stop=True)
                    h3 = hpool.tile([128, N], st_dt, tag="h3")
                    if sti % 2 == 0:
                        nc.scalar.activation(h3[:], p3[:], relu,
                                             bias=bias(t, 4))
                    else:
                        nc.vector.tensor_scalar(
                            h3[:], p3[:], bias(t, 4), 0.0,
                            mybir.AluOpType.add, mybir.AluOpType.max)

                    # per-atom head: [1, N] energy row (row 1 is zeros)
                    po = popool.tile([2, N], _F32, tag="po")
                    nc.tensor.matmul(po[:], wo2(t), h3[:],
                                     start=True, stop=True)
                    lj = ccol // PTILE
                    nc.scalar.copy(
                        o_view[:, lj:lj + nt, :],
                        po[0:1, :].rearrange("o (j p) -> o j p", p=128))
                    ccol += N

                if "nored" not in KDBG:
                    if pending is not None:
                        reduce_chunk(*pending)
                    pending = (o_chunk, jg, ctiles)
                jg += ctiles
                col += cN
            if pending is not None:
                reduce_chunk(*pending)

            # ---- final ----
            if KRED == "bmm":
                res_sb = cpool.tile([1, B], _F32, tag="res")
                if "nored" in KDBG:
                    nc.vector.memset(res_sb[:], 0.0)
                else:
                    nc.scalar.copy(res_sb[:], pres[:])
            else:
                res_sb = cpool.tile([B, 1], _F32, tag="res")
                if KDBG & {"nored", "nofinal"}:
                    nc.vector.memset(res_sb[:], 0.0)
                else:
                    presf = prespool.tile([B, 1], _F32, tag="pres")
                    final = pcols[(n_chunks - 1) % 2]
                    nc.tensor.matmul(presf[:], final[:], ones_col,
                                     start=True, stop=True)
                    nc.scalar.copy(res_sb[:], presf[:])
            nc.sync.dma_start(out_d[:], res_sb[:])
    nc.finalize()
    return nc


def kernel(**inputs):
    global LAST_RESULTS
    f1 = np.ascontiguousarray(np.asarray(inputs["frag1_layer"], np.float32))
    f2 = np.ascontiguousarray(np.asarray(inputs["frag2_layer"], np.float32))
    cx = np.ascontiguousarray(np.asarray(inputs["complex_layer"], np.float32))
    z1 = np.asarray(inputs["frag1_z"])
    z2 = np.asarray(inputs["frag2_z"])
    zc = np.asarray(inputs["complex_z"])
    W0 = np.asarray(inputs["W0"], np.float32)
    b0 = np.asarray(inputs["b0"], np.float32)
    W1 = np.asarray(inputs["W1"], np.float32)
    b1 = np.asarray(inputs["b1"], np.float32)
    W2 = np.asarray(inputs["W2"], np.float32)
    b2 = np.asarray(inputs["b2"], np.float32)
    Wout = np.asarray(inputs["Wout"], np.float32)
    bout = np.asarray(inputs["bout"], np.float32)

    _st_dt, np_dt = _dtypes()

    x_all = np.concatenate([f1, f2, cx], axis=1)          # [B, Na, F]
    z_all = np.concatenate([z1, z2, zc], axis=1)          # [B, Na]
    Bn, Na, _ = x_all.shape
    assert Bn == B
    sgn = np.concatenate([
        np.full(f1.shape[1], -1.0, np.float32),
        np.full(f2.shape[1], -1.0, np.float32),
        np.full(cx.shape[1], 1.0, np.float32),
    ])

    xf = x_all.reshape(-1, F)
    zf = z_all.reshape(-1).astype(np.int64)
    bidx = np.repeat(np.arange(B), Na)
    sf = np.tile(sgn, B)

    order = np.argsort(zf, kind="stable")
    counts = np.bincount(zf, minlength=T)[:T]
    GRAN = NCORES * PTILE
    padded = -(-counts // GRAN) * GRAN
    k_t = (padded // GRAN).astype(int)
    n_core = int(padded.sum()) // NCORES
    ntt = n_core // PTILE

    # Per-core atom index lists; -1 marks padding (pads land on core 7's
    # tail of each type chunk — compute is identical on every core).
    per_core = [[] for _ in range(NCORES)]
    pos = 0
    for t in range(T):
        ct, pt = int(counts[t]), int(padded[t])
        idx = order[pos:pos + ct]
        pos += ct
        if pt == 0:
            continue
        ip = np.full(pt, -1, np.int64)
        ip[:ct] = idx
        ip = ip.reshape(NCORES, pt // NCORES)
        for c in range(NCORES):
            per_core[c].append(ip[c])
    idx_cores = np.stack([np.concatenate(l) for l in per_core])  # [NC, n]

    valid = idx_cores >= 0
    safe = np.where(valid, idx_cores, 0)
    xg = xf[safe]
    xg[~valid] = 0.0
    xT = np.ascontiguousarray(xg.transpose(0, 2, 1)).astype(np_dt)  # [NC,F,n]

    # S[c, n, b] = sign * (batch == b)
    S = np.zeros((NCORES, n_core, B), np.float32)
    rows = sf[safe] * valid
    bcols = bidx[safe]
    S[np.arange(NCORES)[:, None], np.arange(n_core)[None, :], bcols] = rows

    # CONSTW: weights packed per type in the _TBLK layout
    CWh = np.zeros((PTILE, _WCOLS), np.float32)
    for t in range(T):
        base = t * _TBLK
        for k in range(2):
            for m in range(2):
                CWh[:, base + (k * 2 + m) * 128:base + (k * 2 + m + 1) * 128] = \
                    W0[t, 128 * k:128 * (k + 1), 128 * m:128 * (m + 1)]
                CWh[:, base + 512 + (k * 2 + m) * 128:
                    base + 512 + (k * 2 + m + 1) * 128] = \
                    W1[t, 128 * k:128 * (k + 1), 128 * m:128 * (m + 1)]
            CWh[:, base + 1024 + k * 128:base + 1024 + (k + 1) * 128] = \
                W2[t, 128 * k:128 * (k + 1), 0:128]
        CWh[:, base + 1280] = Wout[t, :, 0]
    CWh = np.ascontiguousarray(CWh).astype(np_dt)

    # CONSTF: 25 bias cols + ones col (+ b-major S2 in ttr0 mode)
    CFh = np.zeros((PTILE, 26), np.float32)
    for t in range(T):
        CFh[:, t * 5 + 0] = b0[t, :128]
        CFh[:, t * 5 + 1] = b0[t, 128:]
        CFh[:, t * 5 + 2] = b1[t, :128]
        CFh[:, t * 5 + 3] = b1[t, 128:]
        CFh[:, t * 5 + 4] = b2[t, :128]
    CFh[:, 25] = 1.0

    bias_term = np.bincount(bidx, weights=(sf * bout[zf, 0]).astype(np.float64),
                            minlength=B)[:B]

    nc = _build(k_t, n_core)
    in_maps = []
    for c in range(NCORES):
        m = {"xT": xT[c], "CONSTW": CWh}
        if KRED == "bmm":
            m["CONSTF"] = CFh
            # S2[p, j*B + b] (tile-major)
            m["S2"] = np.ascontiguousarray(
                S[c].reshape(ntt, PTILE, B).transpose(1, 0, 2)
            ).reshape(PTILE, ntt * B).astype(np_dt)
        else:
            # b-major: S2[p, b*ntt + j]
            s2b = np.ascontiguousarray(
                S[c].reshape(ntt, PTILE, B).transpose(1, 2, 0)
            ).reshape(PTILE, B * ntt)
            m["CONSTF"] = np.ascontiguousarray(
                np.concatenate([CFh, s2b], axis=1))
        in_maps.append(m)

    kw = {}
    if TRACE:
        kw = dict(trace=True, trace_cores=list(range(NCORES)))
    res = run_bass_kernel_spmd(nc, in_maps, core_ids=list(range(NCORES)), **kw)
    LAST_RESULTS = res

    parts = np.stack([res.results[c]["res"].reshape(B).astype(np.float64)
                      for c in range(NCORES)])
    out = parts.sum(axis=0) + bias_term
    return out.astype(np.float32)[:, None]
